# revision 1
# baseline (speedup 1.0000x reference)
"""Llama decoder layer on 8 Trainium2 NeuronCores (tensor-parallel).

Sharding (per core c):
  - QKV: q heads {2c, 2c+1} (256 cols), kv head c//2 (128+128 cols), computed
    from the full sequence in transposed layout.
  - attention: 2 q heads per core, full causal sequence, flash-style in SBUF.
  - o_proj: row shard (256 rows) -> partial [4096, 2048], chunked
    ReduceScatter(add) -> each core owns 64 rows per 512-row chunk.
  - MLP: gate/up column shard (704+704, 64-interleaved), down row shard (bf16),
    chunked ReduceScatter after down.
  - RMSNorm weights folded into the following matmul weights on the host;
    1/sqrt(head_dim) folded into Wq.

Layout strategy: activations live transposed ([feature, seq]) so every matmul
is PE-friendly (contraction on partitions); softmax runs on S^T tiles with
denominators via ones-matmuls; attention output emerges transposed, which is
exactly the lhsT layout o_proj needs.
"""
import sys
sys.path.insert(0, "/opt/trn_rl_repo")

import numpy as np

import os
import concourse.bass as bass
import concourse.mybir as mybir
import concourse.tile as tile
from concourse import bacc
from concourse.masks import make_identity

F32 = mybir.dt.float32
F32R = mybir.dt.float32r
BF16 = mybir.dt.bfloat16
AF = mybir.ActivationFunctionType
ALU = mybir.AluOpType

NCORES = 8
SEQ = 4096
HID = 2048
NH = 16
NKV = 4
HD = 128
INTER = 5632
EPS = 1e-5
THETA = 10000.0

ABLATE = set(os.environ.get("KERNEL_ABLATE", "").split(","))

QH = NH // NCORES            # q heads per core = 2
ISH = INTER // NCORES        # intermediate shard = 704
SCH = 512                    # seq chunk
NCH = SEQ // SCH             # 8 chunks
RROWS = SCH // NCORES        # rows per core per chunk after RS = 64
HB = HID // 128              # 16 hidden blocks
GU_CB = 2 * ISH // 128       # 11 gate/up column blocks per core
DN_KB = (ISH + 127) // 128   # 6 down contraction blocks (last is 64)
NSL = HID // SCH             # 4 column slices of a [64, 2048] row block


def _build():
    nc = bacc.Bacc(None, num_devices=NCORES)

    hiddenT = nc.dram_tensor("hiddenT", [HID, SEQ], F32, kind="ExternalInput")
    hid_res = nc.dram_tensor("hid_res", [NCH, RROWS, HID], F32, kind="ExternalInput")
    cos_t = nc.dram_tensor("cos_t", [128, SEQ], F32, kind="ExternalInput")
    sin_t = nc.dram_tensor("sin_t", [128, SEQ], F32, kind="ExternalInput")
    w_qkv_s = nc.dram_tensor("w_qkv_s", [HID, (QH + 2) * HD], F32, kind="ExternalInput")
    w_o_s = nc.dram_tensor("w_o_s", [QH * HD, HID], F32, kind="ExternalInput")
    w_gu_s = nc.dram_tensor("w_gu_s", [HID, 2 * ISH], F32, kind="ExternalInput")
    w_dn_s = nc.dram_tensor("w_dn_s", [ISH, HID], BF16, kind="ExternalInput")
    out = nc.dram_tensor("out", [NCH, RROWS, HID], F32, kind="ExternalOutput")

    rg = [list(range(NCORES))]

    with tile.TileContext(nc) as tc:
        _emit(nc, tc, hiddenT, hid_res, cos_t, sin_t,
              w_qkv_s, w_o_s, w_gu_s, w_dn_s, out, rg)
    nc.finalize()
    return nc


def _emit(nc, tc, hiddenT, hid_res, cos_t, sin_t,
          w_qkv_s, w_o_s, w_gu_s, w_dn_s, out, rg):
    from contextlib import ExitStack
    es = ExitStack()

    # ---------------- constants ----------------
    const = es.enter_context(tc.tile_pool(name="const", bufs=1))
    ident32 = const.tile([128, 128], F32, name="ident32")
    make_identity(nc, ident32)
    ident = const.tile([128, 128], F32R, name="ident")
    nc.vector.tensor_copy(ident[:], ident32[:])
    ones32 = const.tile([128, 1], F32, name="ones32")
    nc.vector.memset(ones32[:], 1.0)
    ones = const.tile([128, 1], F32R, name="ones")
    nc.vector.tensor_copy(ones[:], ones32[:])
    epsc = const.tile([128, 1], F32, name="epsc")
    nc.vector.memset(epsc[:], EPS)
    # causal masks for the 4 diagonal t-blocks of a 512-wide q chunk:
    # mask_j[p, col] = 1.0 if col - j*128 - p >= 0 else 0.0
    masks = []
    for j in range(4):
        m32 = const.tile([128, SCH], F32, name="m32scratch", tag="m32scratch")
        nc.vector.memset(m32[:], 1.0)
        nc.gpsimd.affine_select(
            out=m32[:], in_=m32[:], compare_op=ALU.is_ge,
            fill=0.0, base=-j * 128, channel_multiplier=-1, pattern=[[1, SCH]],
        )
        mj = const.tile([128, SCH], F32R, name=f"mask_{j}")
        nc.vector.tensor_copy(mj[:], m32[:])
        masks.append(mj)

    # ---------------- PSUM pools (8 banks total) ----------------
    ps_mm = es.enter_context(tc.tile_pool(name="ps_mm", bufs=1, space="PSUM"))
    ps_ss = es.enter_context(tc.tile_pool(name="ps_ss", bufs=1, space="PSUM"))
    ps_tr = es.enter_context(tc.tile_pool(name="ps_tr", bufs=1, space="PSUM"))
    ps_s = es.enter_context(tc.tile_pool(name="ps_s", bufs=2, space="PSUM"))
    ps_o = es.enter_context(tc.tile_pool(name="ps_o", bufs=1, space="PSUM"))
    ps_den = es.enter_context(tc.tile_pool(name="ps_den", bufs=1, space="PSUM"))
    ps_op = es.enter_context(tc.tile_pool(name="ps_op", bufs=1, space="PSUM"))

    # ---------------- DRAM scratch ----------------
    dr_rs1in = es.enter_context(tc.tile_pool(name="dr_rs1in", bufs=2, space="DRAM"))
    dr_rs1out = es.enter_context(tc.tile_pool(name="dr_rs1out", bufs=2, space="DRAM"))
    dr_agin = es.enter_context(tc.tile_pool(name="dr_agin", bufs=2, space="DRAM"))
    dr_agout = es.enter_context(tc.tile_pool(name="dr_agout", bufs=NCH, space="DRAM"))
    dr_h2 = es.enter_context(tc.tile_pool(name="dr_h2", bufs=NCH, space="DRAM"))
    dr_rs2in = es.enter_context(tc.tile_pool(name="dr_rs2in", bufs=2, space="DRAM"))
    dr_rs2out = es.enter_context(tc.tile_pool(name="dr_rs2out", bufs=2, space="DRAM"))

    # small shared work pool (tiny [*, 1] and [1, *] tiles)
    wk = es.enter_context(tc.tile_pool(name="wk", bufs=4))

    ag_outs = []
    h2ds = []

    ab = ExitStack()
    # ---------------- persistent attention tensors (phases A+B only) ----
    att = ab.enter_context(tc.tile_pool(name="att", bufs=1))
    qT = [att.tile([128, SEQ], F32R, name=f"qT{h}") for h in range(QH)]
    kT = att.tile([128, SEQ], F32R, name="kT")
    vN = att.tile([128, SEQ], F32R, name="vN")  # V natural; t-block tb at cols tb*128

    # ============ Phase A: QKV projection + RMSNorm + RoPE ============
    with tc.tile_pool(name="wqkv", bufs=1) as wqkvp, \
         tc.tile_pool(name="trig", bufs=2) as trigp, \
         tc.tile_pool(name="xp", bufs=17) as xp, \
         tc.tile_pool(name="aw", bufs=2) as aw, \
         tc.tile_pool(name="rw", bufs=2) as rw:

        wqkv_sb = wqkvp.tile([128, HB * 512], F32R, name="wqkv_sb")
        for hb in range(HB):
            nc.sync.dma_start(wqkv_sb[:, hb * 512:(hb + 1) * 512],
                              w_qkv_s[hb * 128:(hb + 1) * 128, :].bitcast(F32R))

        for sc in range(NCH):
            scol = slice(sc * SCH, (sc + 1) * SCH)
            cos_sb = trigp.tile([128, SCH], F32, name="cos_sb", tag="cos_sb")
            sin_sb = trigp.tile([128, SCH], F32, name="sin_sb", tag="sin_sb")
            nc.sync.dma_start(cos_sb[:], cos_t[:, scol])
            nc.sync.dma_start(sin_sb[:], sin_t[:, scol])
            xts = []
            for hb in range(HB):
                xt = xp.tile([128, SCH], F32R, name="xt", tag="xt")
                nc.sync.dma_start(xt[:], hiddenT[hb * 128:(hb + 1) * 128, scol].bitcast(F32R))
                xts.append(xt)

            # sum of squares over hidden dim (per seq column) via ones-matmul
            ss_ps = ps_ss.tile([1, SCH], F32, name="ss_ps")
            for hb in range(HB):
                sq = aw.tile([128, SCH], F32R, name="sq", tag="sq")
                nc.vector.tensor_mul(sq[:], xts[hb].bitcast(F32)[:], xts[hb].bitcast(F32)[:])
                nc.tensor.matmul(ss_ps[:], ones[:], sq[:],
                                 start=(hb == 0), stop=(hb == HB - 1),
                                 skip_group_check=True)
            stdv = wk.tile([1, SCH], F32, name="stdv", tag="stdv")
            nc.scalar.activation(stdv[:], ss_ps[:], AF.Sqrt, scale=1.0 / HID,
                                 bias=epsc[0:1, :])
            rinv = wk.tile([1, SCH], F32, name="rinv", tag="rinv")
            nc.vector.reciprocal(rinv[:], stdv[:])
            rinv_bc = aw.tile([128, SCH], F32, name="rinv_bc", tag="rinv_bc")
            nc.gpsimd.partition_broadcast(rinv_bc[:], rinv[:])

            # qkv matmuls: c-block outer, hidden-block accumulation inner
            for cb in range(QH + 2):
                qkv_ps = ps_mm.tile([128, SCH], F32, name="qkv_ps")
                for hb in range(HB):
                    nc.tensor.matmul(qkv_ps[:],
                                     wqkv_sb[:, hb * 512 + cb * 128:hb * 512 + (cb + 1) * 128],
                                     xts[hb][:],
                                     start=(hb == 0), stop=(hb == HB - 1),
                                     skip_group_check=True)
                raw = rw.tile([128, SCH], F32, name="raw", tag="raw")
                nc.vector.tensor_mul(raw[:], qkv_ps[:], rinv_bc[:])
                if cb < QH + 1:
                    # rope into qT[cb] or kT:
                    # dst = raw*cosF + swap(raw)*sinF, sinF rows 0-63 negated
                    dst = qT[cb] if cb < QH else kT
                    swp = rw.tile([128, SCH], F32, name="swp", tag="swp")
                    nc.sync.dma_start(swp[0:64, :], raw[64:128, :])
                    nc.sync.dma_start(swp[64:128, :], raw[0:64, :])
                    t1 = rw.tile([128, SCH], F32, name="t1", tag="t1")
                    t2 = rw.tile([128, SCH], F32, name="t2", tag="t2")
                    nc.vector.tensor_mul(t1[:], raw[:], cos_sb[:])
                    nc.vector.tensor_mul(t2[:], swp[:], sin_sb[:])
                    nc.vector.tensor_add(dst[:, scol], t1[:], t2[:])
                else:
                    # V: transpose [d, s-chunk] -> natural [t, d] blocks
                    rawr = rw.tile([128, SCH], F32R, name="rawr", tag="rawr")
                    nc.vector.tensor_copy(rawr[:], raw[:])
                    for i in range(SCH // 128):
                        tp = ps_tr.tile([128, 128], F32R, name="tp", tag="tp")
                        nc.tensor.transpose(tp[:], rawr[:, i * 128:(i + 1) * 128], ident[:])
                        nc.vector.tensor_copy(vN[:, (sc * 4 + i) * 128:(sc * 4 + i + 1) * 128],
                                              tp.bitcast(F32)[:])

    # ============ Phase B+C: attention, o_proj, RS1, ln2, AG ============
    with tc.tile_pool(name="wo", bufs=1) as wop, \
         tc.tile_pool(name="ew", bufs=4) as ew, \
         tc.tile_pool(name="atw", bufs=3) as atw, \
         tc.tile_pool(name="ow", bufs=3) as ow, \
         tc.tile_pool(name="cw", bufs=2) as cw, \
         tc.tile_pool(name="h2p", bufs=5) as h2p:

        wo_sb = wop.tile([128, QH * HID], F32R, name="wo_sb")
        for h in range(QH):
            nc.sync.dma_start(wo_sb[:, h * HID:(h + 1) * HID],
                              w_o_s[h * 128:(h + 1) * 128, :].bitcast(F32R))

        for qc in range(NCH):
            scol = slice(qc * SCH, (qc + 1) * SCH)
            ntb = 4 * qc + 4
            attnT = []
            for h in range(QH):
                if "noattn" in ABLATE:
                    aT = atw.tile([128, SCH], F32R, name="aT", tag="aT")
                    nc.vector.memset(aT.bitcast(F32)[:], 0.001)
                    attnT.append(aT)
                    continue
                o_ps = ps_o.tile([128, SCH], F32, name="o_ps")
                den_ps = ps_den.tile([1, SCH], F32, name="den_ps")
                for tb in range(ntb):
                    s_ps = ps_s.tile([128, SCH], F32, name="s_ps")
                    nc.tensor.matmul(s_ps[:], kT[:, tb * 128:(tb + 1) * 128],
                                     qT[h][:, scol], start=True, stop=True,
                                     skip_group_check=True)
                    eT = ew.tile([128, SCH], F32R, name="eT", tag="eT")
                    nc.scalar.activation(eT[:], s_ps[:], AF.Exp)
                    j = tb - 4 * qc
                    if j >= 0:
                        eTm = ew.tile([128, SCH], F32R, name="eTm", tag="eTm")
                        nc.vector.tensor_mul(eTm[:], eT.bitcast(F32)[:],
                                             masks[j].bitcast(F32)[:])
                        eT = eTm
                    nc.tensor.matmul(o_ps[:], vN[:, tb * 128:(tb + 1) * 128], eT[:],
                                     start=(tb == 0), stop=(tb == ntb - 1),
                                     skip_group_check=True)
                    nc.tensor.matmul(den_ps[:], ones[:], eT[:],
                                     start=(tb == 0), stop=(tb == ntb - 1),
                                     skip_group_check=True)
                dinv = wk.tile([1, SCH], F32, name="dinv", tag="dinv")
                nc.vector.reciprocal(dinv[:], den_ps[:])
                dinv_bc = atw.tile([128, SCH], F32, name="dinv_bc", tag="dinv_bc")
                nc.gpsimd.partition_broadcast(dinv_bc[:], dinv[:])
                aT = atw.tile([128, SCH], F32R, name="aT", tag="aT")
                nc.vector.tensor_mul(aT[:], o_ps[:], dinv_bc[:])
                attnT.append(aT)

            # o_proj for this chunk -> rs1 input
            rs1_in = dr_rs1in.tile([SCH, HID], F32, name="rs1_in", tag="rs1_in")
            for sb in range(SCH // 128):
                for nch_ in range(HID // 512):
                    op_ps = ps_op.tile([128, 512], F32, name="op_ps")
                    for h in range(QH):
                        nc.tensor.matmul(op_ps[:],
                                         attnT[h][:, sb * 128:(sb + 1) * 128],
                                         wo_sb[:, h * HID + nch_ * 512:h * HID + (nch_ + 1) * 512],
                                         start=(h == 0), stop=(h == QH - 1),
                                         skip_group_check=True)
                    ot = ow.tile([128, 512], F32, name="ot", tag="ot")
                    nc.vector.tensor_copy(ot[:], op_ps[:])
                    nc.sync.dma_start(rs1_in[sb * 128:(sb + 1) * 128, nch_ * 512:(nch_ + 1) * 512], ot[:])

            rs1_out = dr_rs1out.tile([RROWS, HID], F32, name="rs1_out", tag="rs1_out")
            if "nocoll" in ABLATE:
                nc.sync.dma_start(rs1_out[:], rs1_in[0:RROWS, :])
            else:
                nc.gpsimd.collective_compute(
                    "ReduceScatter", ALU.add, replica_groups=rg,
                    ins=[rs1_in[:].opt()], outs=[rs1_out[:].opt()])

            # residual + ln2 on own 64 rows, in 4 column slices of 512
            h2d = dr_h2.tile([RROWS, HID], F32, name="h2d", tag="h2d")
            h2s = []
            ss2p = wk.tile([RROWS, NSL], F32, name="ss2p", tag="ss2p")
            for sl in range(NSL):
                csl = slice(sl * SCH, (sl + 1) * SCH)
                acc = cw.tile([RROWS, SCH], F32, name="acc", tag="acc")
                nc.sync.dma_start(acc[:], rs1_out[:, csl])
                res = cw.tile([RROWS, SCH], F32, name="res", tag="res")
                nc.sync.dma_start(res[:], hid_res[qc][:, csl])
                h2 = h2p.tile([RROWS, SCH], F32, name="h2", tag="h2")
                nc.vector.tensor_add(h2[:], acc[:], res[:])
                nc.sync.dma_start(h2d[:, csl], h2[:])
                sq2 = cw.tile([RROWS, SCH], F32, name="sq2", tag="sq2")
                nc.vector.scalar_tensor_tensor(
                    out=sq2[:], in0=h2[:], scalar=1.0, in1=h2[:],
                    op0=ALU.mult, op1=ALU.mult, accum_out=ss2p[:, sl:sl + 1])
                h2s.append(h2)
            ss2 = wk.tile([RROWS, 1], F32, name="ss2", tag="ss2")
            nc.vector.tensor_reduce(ss2[:], ss2p[:], axis=mybir.AxisListType.X, op=ALU.add)
            std2 = wk.tile([RROWS, 1], F32, name="std2", tag="std2")
            nc.scalar.activation(std2[:], ss2[:], AF.Sqrt, scale=1.0 / HID,
                                 bias=epsc[0:RROWS, :])
            rinv2 = wk.tile([RROWS, 1], F32, name="rinv2", tag="rinv2")
            nc.vector.reciprocal(rinv2[:], std2[:])

            # xn2 slices -> transpose -> ag_in [HID, 64]
            ag_in = dr_agin.tile([HID, RROWS], F32, name="ag_in", tag="ag_in")
            for sl in range(NSL):
                xn2 = cw.tile([RROWS, SCH], F32R, name="xn2", tag="xn2")
                nc.scalar.activation(xn2[:], h2s[sl][:], AF.Copy, scale=rinv2[:])
                for k in range(SCH // 128):
                    hb = sl * (SCH // 128) + k
                    tp = ps_tr.tile([128, RROWS], F32R, name="tp2", tag="tp")
                    nc.tensor.transpose(tp[:], xn2[:, k * 128:(k + 1) * 128],
                                        ident[0:RROWS, 0:RROWS])
                    tpc = ow.tile([128, RROWS], F32, name="tpc", tag="tpc")
                    nc.vector.tensor_copy(tpc[:], tp.bitcast(F32)[:])
                    nc.sync.dma_start(ag_in[hb * 128:(hb + 1) * 128, :], tpc[:])
            ag_out = dr_agout.tile([NCORES * HID, RROWS], F32, name="ag_out", tag="ag_out",
                                   addr_space="Local" if "nocoll" in ABLATE else "Shared")
            if "nocoll" in ABLATE:
                for r in range(NCORES):
                    nc.sync.dma_start(ag_out[r * HID:(r + 1) * HID, :], ag_in[:])
            else:
                nc.gpsimd.collective_compute(
                    "AllGather", ALU.bypass, replica_groups=rg,
                    ins=[ag_in[:].opt()], outs=[ag_out[:].opt()])
            ag_outs.append(ag_out)
            h2ds.append(h2d)

    ab.close()

    # ============ Phase D: MLP + RS2 + final residual ============
    with tc.tile_pool(name="wgu", bufs=1) as wgup, \
         tc.tile_pool(name="wdn", bufs=1) as wdnp, \
         tc.tile_pool(name="mx", bufs=17) as mx, \
         tc.tile_pool(name="hTp", bufs=1) as hTp, \
         tc.tile_pool(name="dw", bufs=2) as dw:

        wgu_sb = wgup.tile([128, HB * 2 * ISH], F32R, name="wgu_sb")
        for hb in range(HB):
            nc.sync.dma_start(wgu_sb[:, hb * 2 * ISH:(hb + 1) * 2 * ISH],
                              w_gu_s[hb * 128:(hb + 1) * 128, :].bitcast(F32R))
        wdn_sb = wdnp.tile([128, DN_KB * HID], BF16, name="wdn_sb")
        for m in range(DN_KB):
            rows = min(128, ISH - m * 128)
            nc.sync.dma_start(wdn_sb[0:rows, m * HID:(m + 1) * HID],
                              w_dn_s[m * 128:m * 128 + rows, :])

        for c in range(NCH):
            # ag_out rows are [rank, hid]-major; view as [h, rank, s] per h-block
            ag3 = ag_outs[c][:].rearrange("(r h) s -> h r s", r=NCORES)
            xts = []
            for hb in range(HB):
                xt = mx.tile([128, SCH], F32R, name="mxt", tag="mxt")
                nc.sync.dma_start(xt[:].rearrange("p (r s) -> p r s", r=NCORES),
                                  ag3[hb * 128:(hb + 1) * 128, :, :].bitcast(F32R))
                xts.append(xt)

            if "nomlp" in ABLATE:
                for sl in range(NSL):
                    csl = slice(sl * SCH, (sl + 1) * SCH)
                    m_in = dw.tile([RROWS, SCH], F32, name="m_in", tag="m_in")
                    nc.sync.dma_start(m_in[:], h2ds[c][:, csl])
                    nc.sync.dma_start(out[c][:, csl], m_in[:])
                continue
            hT = hTp.tile([128, DN_KB * SCH], BF16, name="hT", tag="hT")
            for m in range(DN_KB):
                rows = min(128, ISH - m * 128)
                gcol = 256 * m
                ucol = 256 * m + rows
                g_ps = ps_s.tile([128, SCH], F32, name="g_ps", tag="s_ps")
                for hb in range(HB):
                    nc.tensor.matmul(g_ps[0:rows, :],
                                     wgu_sb[:, hb * 2 * ISH + gcol:hb * 2 * ISH + gcol + rows],
                                     xts[hb][:],
                                     start=(hb == 0), stop=(hb == HB - 1),
                                     skip_group_check=True)
                u_ps = ps_s.tile([128, SCH], F32, name="u_ps", tag="s_ps")
                for hb in range(HB):
                    nc.tensor.matmul(u_ps[0:rows, :],
                                     wgu_sb[:, hb * 2 * ISH + ucol:hb * 2 * ISH + ucol + rows],
                                     xts[hb][:],
                                     start=(hb == 0), stop=(hb == HB - 1),
                                     skip_group_check=True)
                sg = dw.tile([128, SCH], F32, name="sg", tag="sg")
                nc.scalar.activation(sg[0:rows, :], g_ps[0:rows, :], AF.Silu)
                nc.vector.tensor_mul(hT[0:rows, m * SCH:(m + 1) * SCH],
                                     sg[0:rows, :], u_ps[0:rows, :])

            rs2_in = dr_rs2in.tile([SCH, HID], F32, name="rs2_in", tag="rs2_in")
            for sb in range(SCH // 128):
                for nch_ in range(HID // 512):
                    dn_ps = ps_o.tile([128, 512], F32, name="dn_ps", tag="o_ps")
                    for m in range(DN_KB):
                        rows = min(128, ISH - m * 128)
                        nc.tensor.matmul(dn_ps[:],
                                         hT[0:rows, m * SCH + sb * 128:m * SCH + (sb + 1) * 128],
                                         wdn_sb[0:rows, m * HID + nch_ * 512:m * HID + (nch_ + 1) * 512],
                                         start=(m == 0), stop=(m == DN_KB - 1),
                                         skip_group_check=True)
                    dt_ = dw.tile([128, 512], F32, name="dt_", tag="dt_")
                    nc.vector.tensor_copy(dt_[:], dn_ps[:])
                    nc.sync.dma_start(rs2_in[sb * 128:(sb + 1) * 128, nch_ * 512:(nch_ + 1) * 512], dt_[:])

            rs2_out = dr_rs2out.tile([RROWS, HID], F32, name="rs2_out", tag="rs2_out")
            if "nocoll" in ABLATE:
                nc.sync.dma_start(rs2_out[:], rs2_in[0:RROWS, :])
            else:
                nc.gpsimd.collective_compute(
                    "ReduceScatter", ALU.add, replica_groups=rg,
                    ins=[rs2_in[:].opt()], outs=[rs2_out[:].opt()])
            for sl in range(NSL):
                csl = slice(sl * SCH, (sl + 1) * SCH)
                m_in = dw.tile([RROWS, SCH], F32, name="m_in", tag="m_in")
                nc.sync.dma_start(m_in[:], rs2_out[:, csl])
                h2b = dw.tile([RROWS, SCH], F32, name="h2b", tag="h2b")
                nc.sync.dma_start(h2b[:], h2ds[c][:, csl])
                nc.vector.tensor_add(m_in[:], m_in[:], h2b[:])
                nc.sync.dma_start(out[c][:, csl], m_in[:])

    es.close()


# ---------------- host side ----------------

_CACHE = {}


def _get_runner():
    if "runner" in _CACHE:
        return _CACHE["runner"]
    import jax
    from jax.sharding import Mesh, PartitionSpec
    from jax.experimental.shard_map import shard_map
    from concourse import bass2jax

    nc = _build()
    bass2jax.install_neuronx_cc_hook()

    in_names = []
    out_names = []
    out_avals = []
    zero_shapes = []
    for alloc in nc.m.functions[0].allocations:
        if not isinstance(alloc, mybir.MemoryLocationSet):
            continue
        name = alloc.memorylocations[0].name
        if alloc.kind == "ExternalInput":
            if nc.partition_id_tensor is None or name != nc.partition_id_tensor.name:
                in_names.append(name)
        elif alloc.kind == "ExternalOutput":
            out_names.append(name)
            shape = tuple(alloc.tensor_shape)
            dtype = mybir.dt.np(alloc.dtype)
            out_avals.append(jax.core.ShapedArray(shape, dtype))
            zero_shapes.append((shape, dtype))
    n_params = len(in_names)
    full_in_names = list(in_names) + list(out_names)
    if nc.partition_id_tensor is not None:
        full_in_names.append(nc.partition_id_tensor.name)
    donate = tuple(range(n_params, n_params + len(out_names)))

    def _body(*args):
        operands = list(args)
        if nc.partition_id_tensor is not None:
            operands.append(bass2jax.partition_id_tensor())
        outs = bass2jax._bass_exec_p.bind(
            *operands,
            out_avals=tuple(out_avals),
            in_names=tuple(full_in_names),
            out_names=tuple(out_names),
            lowering_input_output_aliases=(),
            sim_require_finite=True,
            sim_require_nnan=True,
            nc=nc,
        )
        return tuple(outs)

    devices = jax.devices()[:NCORES]
    mesh = Mesh(np.asarray(devices), ("core",))
    in_specs = (PartitionSpec("core"),) * (n_params + len(out_names))
    out_specs = (PartitionSpec("core"),) * len(out_names)
    sharded = jax.jit(
        shard_map(_body, mesh=mesh, in_specs=in_specs, out_specs=out_specs,
                  check_rep=False),
        donate_argnums=donate, keep_unused=True,
    )
    runner = dict(fn=sharded, in_names=in_names, out_names=out_names,
                  zero_shapes=zero_shapes, out_avals=out_avals)
    _CACHE["runner"] = runner
    return runner


def _prep_inputs(positions, hidden_states, ln1_w, ln2_w, w_qkv, w_o, w_gate_up, w_down):
    """Build per-core input dicts (list of NCORES dicts, numpy)."""
    import ml_dtypes
    hs = np.asarray(hidden_states, dtype=np.float32)
    pos = np.asarray(positions, dtype=np.float64)
    ln1 = np.asarray(ln1_w, dtype=np.float32)
    ln2 = np.asarray(ln2_w, dtype=np.float32)
    wq = np.asarray(w_qkv, dtype=np.float32)
    wo = np.asarray(w_o, dtype=np.float32)
    wgu = np.asarray(w_gate_up, dtype=np.float32)
    wdn = np.asarray(w_down, dtype=np.float32)

    hiddenT = np.ascontiguousarray(hs.T)
    inv_freq = 1.0 / (THETA ** (np.arange(0, HD, 2, dtype=np.float64) / HD))
    freqs = pos[:, None] * inv_freq[None, :]          # [SEQ, 64]
    cos_h = np.cos(freqs).T.astype(np.float32)          # [64, SEQ]
    sin_h = np.sin(freqs).T.astype(np.float32)
    cos_t = np.ascontiguousarray(np.concatenate([cos_h, cos_h], axis=0))   # [128, SEQ]
    sin_t = np.ascontiguousarray(np.concatenate([-sin_h, sin_h], axis=0))  # [128, SEQ]

    q_size = NH * HD
    kv_size = NKV * HD
    wq_eff = wq * ln1[:, None]
    wgu_eff = wgu * ln2[:, None]
    scale = HD ** -0.5

    per_core = []
    for c in range(NCORES):
        kvh = c // 2
        q_cols = wq_eff[:, 2 * c * HD:(2 * c + QH) * HD] * scale
        k_cols = wq_eff[:, q_size + kvh * HD:q_size + (kvh + 1) * HD]
        v_cols = wq_eff[:, q_size + kv_size + kvh * HD:q_size + kv_size + (kvh + 1) * HD]
        w_qkv_s = np.ascontiguousarray(np.concatenate([q_cols, k_cols, v_cols], axis=1))
        w_o_s = np.ascontiguousarray(wo[2 * c * HD:(2 * c + QH) * HD, :])
        g = wgu_eff[:, ISH * c:ISH * (c + 1)]
        u = wgu_eff[:, INTER + ISH * c:INTER + ISH * (c + 1)]
        gu_parts = []
        for m in range((ISH + 127) // 128):
            rows = min(128, ISH - m * 128)
            gu_parts.append(g[:, m * 128:m * 128 + rows])
            gu_parts.append(u[:, m * 128:m * 128 + rows])
        w_gu_s = np.ascontiguousarray(np.concatenate(gu_parts, axis=1))
        w_dn_s = np.ascontiguousarray(wdn[ISH * c:ISH * (c + 1), :]).astype(ml_dtypes.bfloat16)
        hid_res = np.ascontiguousarray(
            np.stack([hs[qc * SCH + c * RROWS:qc * SCH + (c + 1) * RROWS] for qc in range(NCH)]))
        per_core.append({
            "hiddenT": hiddenT, "hid_res": hid_res,
            "cos_t": cos_t, "sin_t": sin_t,
            "w_qkv_s": w_qkv_s, "w_o_s": w_o_s,
            "w_gu_s": w_gu_s, "w_dn_s": w_dn_s,
        })
    return per_core


def kernel(positions, hidden_states, ln1_w, ln2_w, w_qkv, w_o, w_gate_up, w_down):
    runner = _get_runner()
    per_core = _prep_inputs(positions, hidden_states, ln1_w, ln2_w,
                            w_qkv, w_o, w_gate_up, w_down)
    concat_in = [
        np.concatenate([np.asarray(per_core[c][name]) for c in range(NCORES)], axis=0)
        for name in runner["in_names"]
    ]
    concat_zeros = [
        np.zeros((NCORES * s[0],) + tuple(s[1:]), d)
        for (s, d) in runner["zero_shapes"]
    ]
    outs = runner["fn"](*concat_in, *concat_zeros)
    out = np.asarray(outs[0]).reshape(NCORES, NCH, RROWS, HID)
    # out[r, qc, i, :] = full[qc*SCH + r*RROWS + i]
    full = out.transpose(1, 0, 2, 3).reshape(SEQ, HID)
    return full


if __name__ == "__main__":
    print("building...")
    _get_runner()
    print("built ok")

